# revision 27
# baseline (speedup 1.0000x reference)
"""Trainium2 Bass kernel for the ragged triangular-GEMM decoder.

Computation (reference): out[b, i, :] = sum_{l<=i} x[b, l, :] @ W_i[l]
with x: [128, 12, 4096] fp32, W_i: [(i+1), 4096, 768] fp32, out: [128, 12, 768].

Decompose the work into 624 units (i, l, u): output layer i, source
layer l <= i, 96-wide output column group u (A=768 -> 8 groups). Each
unit is a [4096f x 96] GEMM contribution. Units are distributed over
8 cores x 5 "slots"; a slot is (one x source chunk l) x (a fixed-width
stack of units of that l). Slot unit-counts [24, 20, 16, 10, 8] sum to
78 = 624/8 and admit an EXACT partition of the ragged triangle (column
l has (12-l)*8 units), so the SPMD program is identical on every core
while each core reads only its 5 x-chunks (5.25 MB fp16) and exactly
1/8 of all weights. Cores emit per-slot partial sums; the host
scatter-adds them into the final output.

W is quantized host-side to fp8 e3m4 (x1024 scale; subnormal-range
values rounded to {0, +-min normal} so PE flush-to-zero behavior is
irrelevant), halving HBM bytes vs fp16. x stays fp16 (the matmul
stationary operand may differ in dtype from the moving operand),
keeping quantization error at ~1.4e-2 rel. PSUM accumulates fp32;
partials drain as fp16 and the host divides by 1024.

Schedule (v2): the kernel is at the PE/DMA ridge (~105 us each), so
wall time is set by how早 the PE starts and whether supply ever lags.
 - PE p-state warmup runs off a MEMSET tile (no DMA dependency), so
   the clock ramps from ~5.8 us while the first real tiles load.
 - W kk-groups ride ONE HWDGE ring (sync) in exact consumption order,
   paced by an 8-deep rotating tag (the prefetch throttle, ~4-14 MB
   ahead, which rides through transient HBM contention from the other
   7 cores). The FIRST slot's x pieces are interleaved on the same
   ring right before the W group that needs them (deterministic
   arrival for the critical start); later slots' x chunks are whole
   sync-ring items inserted mid-previous-slot where supply slack has
   accumulated. (Putting x on the scalar ring does NOT work: HWDGE
   engine arbitration starves the minority ring when the W ring is
   saturated -- measured both as v1's 16.2 us first-x and as a
   regression when all x was scalar-ring-queued up front.)
 - Slots run wide->narrow [0,2,1,3,4]: narrow slots cost the most x
   bytes per PE-second, so they run last when DMA has slack. First
   slot's W groups ramp 2->8 kk so the first matmul starts ~9 us;
   steady groups are 8 kk so a straggling group stalls at most ~2 us.
 - Last slot loops chunk-outer so its first output chunk drains under
   the second chunk's compute; other slots drain via gpsimd SWDGE.
"""

import numpy as np
import ml_dtypes
from contextlib import ExitStack

import concourse.bass as bass
import concourse.tile as tile
from concourse import bacc, mybir
from concourse.bass_utils import run_bass_kernel_spmd

N_CORES = 8
B = 128
L = 12
F = 4096
A = 768
U = 96                      # unit width (output cols)
NU = A // U                 # 8 a-units per layer
KK = F // 128               # 32 k-chunks per source layer

W_SCALE = 1024.0            # host-side fp8 scale (power of 2)

SLOT_N = [24, 20, 16, 10, 8]          # units per slot (sum 78)
SLOT_W = [n * U for n in SLOT_N]      # cols per slot
# wide -> narrow; adjacent slots' PSUM chunks fit 8 banks
# (chunk counts s0:5 s2:3 s1:4 s3:2 s4:2 -> 5+3, 3+4, 4+2, 2+2).
SLOT_ORDER = [0, 2, 1, 3, 4]

# per-slot kk-group splits. First-processed slot ramps up so the first
# matmul's W tile is small; steady-state groups are ~1.5-2.4 MB DMAs
# (8 kk max so a straggling group costs at most ~2.2 us of PE).
SLOT_KK_GROUPS = {
    0: [3, 5, 8, 8, 8],
    2: [8, 8, 8, 8],
    1: [8, 8, 8, 8],
    3: [8, 8, 8, 8],
    4: [8, 8, 8, 8],
}
# First slot's x rides the sync ring in pieces, each emitted right
# before the first W group needing it (deterministic arrival). Later
# slots' x chunks are single sync-ring items inserted mid-previous-slot
# (X_INSERT[s] = (prev_slot, group_idx): emit x_s before that group),
# where the accumulated supply slack absorbs the 1.05 MB dent. An
# attempt to put them on the scalar ring up front regressed badly: the
# HWDGE engine share starved the sync ramp and the first real matmul
# slipped from ~10.5 us to ~17-22 us.
# [5, 27] starts the PE earliest (small first piece). An [8, 24] split
# that removes the xp1 dent between g0 and g1 (a ~2.8 us PE gap) was
# measured ~1.5 us WORSE on mean exec: the later first matmul costs
# more than the gap removal saves -- the ramp is handoff-latency-bound
# and the two trade almost exactly.
SLOT_X_SPLIT = {0: [5, 27]}
# (slot, gi, kk0, kk1): load x[slot][kk0:kk1] right before W group gi of
# the slot at that ring position. Each later slot's x splits 8+24 kk:
# the 8-kk piece a slot early, the 24-kk piece after the owning slot's
# first group (which only needs kk 0-7), spreading the supply dent.
X_INSERT = [
    (2, 0, 3, 0, 8), (2, 2, 1, 8, 32),     # x2: before s0.g3 / s2.g1
    (1, 2, 2, 0, 8), (1, 1, 1, 8, 32),     # x1: before s2.g2 / s1.g1
    (3, 1, 2, 0, 8), (3, 3, 1, 8, 32),     # x3: before s1.g2 / s3.g1
    (4, 3, 2, 0, 8), (4, 4, 1, 8, 32),     # x4: before s3.g2 / s4.g1
]
N_WARMUP_MM = 42            # scratch matmuls to ramp the PE clock
# gpsimd drain order: s3's CAST (~101 us) gates the ring, so all four
# mid-slot output drains (8 MB chip-wide) land AFTER the load crunch
# window (~10-95 us) instead of inside it.
DRAIN_ORDER = [3, 0, 2, 1]

# source chunk l for (slot, core) -- the exact ragged-triangle partition
SLOT_L = [
    [8, 0, 0, 0, 0, 1, 1, 6],    # slot 0: 24 units each
    [5, 2, 2, 2, 1, 1, 3, 3],    # slot 1: 20
    [9, 5, 3, 3, 4, 4, 4, 4],    # slot 2: 16
    [7, 7, 7, 7, 5, 5, 2, 2],    # slot 3: 10
    [11, 10, 10, 9, 8, 6, 6, 6], # slot 4: 8
]

_compiled_nc = None


def _unit_assignment():
    """-> units[(slot, core)] = list of (i, u), exactly SLOT_N[slot] long."""
    pieces_by_l = {l: [] for l in range(L)}
    for s in range(len(SLOT_N)):
        for c in range(N_CORES):
            pieces_by_l[SLOT_L[s][c]].append((s, c))
    out = {}
    for l in range(L):
        units = [(i, u) for i in range(l, L) for u in range(NU)]
        acc = 0
        for (s, c) in sorted(pieces_by_l[l]):
            n = SLOT_N[s]
            out[(s, c)] = units[acc:acc + n]
            acc += n
        assert acc == len(units), (l, acc, len(units))
    return out


def _bounds(sizes: list[int]) -> list[tuple[int, int]]:
    bounds = [0]
    for n in sizes:
        bounds.append(bounds[-1] + n)
    assert bounds[-1] == KK, sizes
    return list(zip(bounds[:-1], bounds[1:]))


def _chunks(w: int) -> list[tuple[int, int]]:
    """Split w cols into <=512-wide PSUM-bank chunks."""
    out = []
    s = 0
    while s < w:
        out.append((s, min(w, s + 512)))
        s += 512
    return out


def _build():
    nc = bacc.Bacc("TRN2", target_bir_lowering=False, debug=False,
                   num_devices=N_CORES)

    xdt = mybir.dt.float16
    wdt = mybir.dt.float8e3
    xs_d = [nc.dram_tensor(f"xs{s}", [128, KK, B], xdt,
                           kind="ExternalInput").ap()
            for s in range(len(SLOT_N))]
    w_d = [nc.dram_tensor(f"w{s}", [128, KK, SLOT_W[s]], wdt,
                          kind="ExternalInput").ap()
           for s in range(len(SLOT_N))]
    out_d = [nc.dram_tensor(f"out{s}", [B, SLOT_W[s]], mybir.dt.float16,
                            kind="ExternalOutput").ap()
             for s in range(len(SLOT_N))]

    s_last = SLOT_ORDER[-1]

    with tile.TileContext(nc) as tc:
        with ExitStack() as ctx:
            xpool = ctx.enter_context(tc.tile_pool(name="x", bufs=1))
            wpool = ctx.enter_context(tc.tile_pool(name="w", bufs=8))
            opool = ctx.enter_context(tc.tile_pool(name="o", bufs=2))
            ppool = ctx.enter_context(tc.tile_pool(name="ps", bufs=8,
                                                   space="PSUM"))
            wupool = ctx.enter_context(tc.tile_pool(name="wu", bufs=1))

            # PE clock warmup off a memset tile: no DMA dependency, so
            # the ramp starts as soon as the preamble ends (~5.8 us).
            warm = wupool.tile([128, 128], wdt, tag="wu", name="warm")
            nc.gpsimd.memset(warm[:], 0.0)
            wps = ppool.tile([128, 128], mybir.dt.float32, tag="pc",
                             name="warm_ps")
            for _ in range(N_WARMUP_MM):
                nc.tensor.matmul(wps[:], warm[:], warm[:],
                                 start=True, stop=True)

            # --- Load emission: one sync HWDGE ring, exact order. ---
            # W kk-groups in consumption order; slot0's x in pieces just
            # before the group needing them; later slots' x as whole
            # chunks inserted mid-previous-slot (X_INSERT).
            s_first = SLOT_ORDER[0]
            xsplit0 = _bounds(SLOT_X_SPLIT[s_first])
            x_at = {}           # (slot, gi) -> [(xslot, kk0, kk1)]
            for xs, ps, gi, kk0, kk1 in X_INSERT:
                x_at.setdefault((ps, gi), []).append((xs, kk0, kk1))
            x_parts = {}        # slot -> [(kk0, kk1, AP)]
            wgs_by_slot = {}
            covered0 = 0
            pi0 = 0
            for s in SLOT_ORDER:
                wgs = []
                for gi, (g0, g1) in enumerate(_bounds(SLOT_KK_GROUPS[s])):
                    if s == s_first:
                        while covered0 < g1:
                            xb0, xb1 = xsplit0[pi0]
                            xp = xpool.tile([128, xb1 - xb0, B], xdt,
                                            tag=f"x{s}p{pi0}", bufs=1,
                                            name=f"x{s}p{pi0}")
                            nc.sync.dma_start(xp[:],
                                              xs_d[s][:, xb0:xb1, :])
                            x_parts.setdefault(s, []).append(
                                (xb0, xb1, xp))
                            covered0 = xb1
                            pi0 += 1
                    for xs, kk0, kk1 in x_at.get((s, gi), []):
                        xp = xpool.tile([128, kk1 - kk0, B], xdt,
                                        tag=f"x{xs}k{kk0}", bufs=1,
                                        name=f"x{xs}k{kk0}")
                        nc.sync.dma_start(xp[:], xs_d[xs][:, kk0:kk1, :])
                        x_parts.setdefault(xs, []).append((kk0, kk1, xp))
                    wg = wpool.tile([128, g1 - g0, SLOT_W[s]], wdt,
                                    tag="wg", name=f"wg{s}_{g0}")
                    nc.sync.dma_start(wg[:], w_d[s][:, g0:g1, :])
                    wgs.append(wg)
                wgs_by_slot[s] = wgs
            assert covered0 == KK and len(x_parts) == len(SLOT_N)
            for s in SLOT_ORDER:
                ks = sorted(x_parts[s])
                assert ks[0][0] == 0 and ks[-1][1] == KK, (s, ks)
                x_parts[s] = ks

            # --- Compute phase, slot by slot. ---
            ots = {}
            for si, s in enumerate(SLOT_ORDER):
                w_cols = SLOT_W[s]
                groups = _bounds(SLOT_KK_GROUPS[s])
                wgs = wgs_by_slot[s]

                def xst(kk, parts=x_parts[s]):
                    for (b0, b1, xp) in parts:
                        if kk < b1:
                            return xp[:, kk - b0, :]
                    raise AssertionError(kk)

                pcs = [ppool.tile([B, c1 - c0], mybir.dt.float32, tag="pc",
                                  name=f"pc{s}_{ci}")
                       for ci, (c0, c1) in enumerate(_chunks(w_cols))]
                # per-slot resident ot tiles (bufs=1) so mid-kernel
                # drains can be deferred past the load-crunch window
                ot = opool.tile([B, w_cols], mybir.dt.float16,
                                tag=f"ot{s}", bufs=1, name=f"ot{s}")

                if s == s_last:
                    # Deferred mid-slot drains: emitted here (before the
                    # last slot's compute) in DRAIN_ORDER; the first is
                    # gated on s3's CAST (~101 us), so all of them land
                    # after the chip-wide load crunch has ended. The
                    # tiny copy below additionally gates the gpsimd ring
                    # on the LAST W group being resident, so on a
                    # contended core the drains can never compete with
                    # the critical final loads.
                    gate = wupool.tile([128, 1], wdt, tag="gate", bufs=1,
                                       name="drain_gate")
                    nc.gpsimd.tensor_copy(gate[:], wgs[-1][:, 0, 0:1])
                    for ds in DRAIN_ORDER:
                        nc.gpsimd.dma_start(out_d[ds][:], ots[ds][:])
                    # chunk-outer: chunk 0 finishes all kk first, drains
                    # (copy + out DMA) while chunk 1 still computes. All
                    # of this slot's W groups stay resident (bufs >= 4).
                    for ci, (c0, c1) in enumerate(_chunks(w_cols)):
                        for gi, (g0, g1) in enumerate(groups):
                            wg = wgs[gi]
                            for kk in range(g0, g1):
                                nc.tensor.matmul(
                                    pcs[ci][:], xst(kk),
                                    wg[:, kk - g0, c0:c1],
                                    start=(kk == 0), stop=(kk == KK - 1),
                                )
                        nc.vector.tensor_copy(ot[:, c0:c1], pcs[ci][:])
                        # ring is drained of loads by now; HWDGE has the
                        # faster first-byte for the critical final drains
                        nc.sync.dma_start(out_d[s][:, c0:c1], ot[:, c0:c1])
                else:
                    for gi, (g0, g1) in enumerate(groups):
                        wg = wgs[gi]
                        for kk in range(g0, g1):
                            for ci, (c0, c1) in enumerate(_chunks(w_cols)):
                                nc.tensor.matmul(
                                    pcs[ci][:], xst(kk),
                                    wg[:, kk - g0, c0:c1],
                                    start=(kk == 0), stop=(kk == KK - 1),
                                )
                    for ci, (c0, c1) in enumerate(_chunks(w_cols)):
                        nc.vector.tensor_copy(ot[:, c0:c1], pcs[ci][:])
                    ots[s] = ot

    nc.compile()
    return nc


def _to_e3m4(a: np.ndarray) -> np.ndarray:
    """fp32 -> e3m4 RTN, with subnormal results re-rounded to {0, +-0.25}
    so hardware FTZ of fp8 subnormal operands cannot change the result."""
    q = np.asarray(a, dtype=ml_dtypes.float8_e3m4)
    qf = q.astype(np.float32)
    sub = np.abs(qf) < 0.25
    if sub.any():
        fix = np.where(np.abs(a) >= 0.125,
                       np.copysign(np.float32(0.25), a),
                       np.float32(0.0)).astype(ml_dtypes.float8_e3m4)
        q = np.where(sub, fix, q)
    return q


def _pack_inputs(x: np.ndarray, Ws: list[np.ndarray]):
    """Host-side shard + layout. Returns (in_maps, assignment)."""
    assign = _unit_assignment()
    # x chunks: xc[l][p, kk, b] = x[b, l, kk*128 + p], fp16
    xc = np.ascontiguousarray(
        x.reshape(B, L, KK, 128).transpose(1, 3, 2, 0)).astype(np.float16)
    in_maps = []
    for c in range(N_CORES):
        m = {}
        for s in range(len(SLOT_N)):
            l = SLOT_L[s][c]
            m[f"xs{s}"] = xc[l]
            parts = []
            for (i, u) in assign[(s, c)]:
                wl = Ws[i][l]  # [F, A] fp32
                blk = wl.reshape(KK, 128, A)[:, :, u * U:(u + 1) * U]
                parts.append(blk.transpose(1, 0, 2))  # [128, KK, 96]
            m[f"w{s}"] = _to_e3m4(
                np.concatenate(parts, axis=2) * np.float32(W_SCALE))
        in_maps.append(m)
    return in_maps, assign


def _run(inputs: dict, trace: bool = False):
    global _compiled_nc
    if _compiled_nc is None:
        _compiled_nc = _build()
    x = np.asarray(inputs["x"], dtype=np.float32)
    Ws = [np.asarray(inputs[f"W_{i}"], dtype=np.float32) for i in range(L)]
    in_maps, assign = _pack_inputs(x, Ws)
    res = run_bass_kernel_spmd(_compiled_nc, in_maps,
                               core_ids=list(range(N_CORES)), trace=trace)
    out = np.zeros((B, L, A), dtype=np.float32)
    for c in range(N_CORES):
        for s in range(len(SLOT_N)):
            part = res.results[c][f"out{s}"].astype(np.float32)
            for k, (i, u) in enumerate(assign[(s, c)]):
                out[:, i, u * U:(u + 1) * U] += part[:, k * U:(k + 1) * U]
    out *= np.float32(1.0 / W_SCALE)
    return out, res


def kernel(**inputs: np.ndarray) -> np.ndarray:
    out, _ = _run(inputs, trace=False)
    return out


# revision 28
# speedup vs baseline: 1.0049x; 1.0049x over previous
"""Trainium2 Bass kernel for the ragged triangular-GEMM decoder.

Computation (reference): out[b, i, :] = sum_{l<=i} x[b, l, :] @ W_i[l]
with x: [128, 12, 4096] fp32, W_i: [(i+1), 4096, 768] fp32, out: [128, 12, 768].

Decompose the work into 624 units (i, l, u): output layer i, source
layer l <= i, 96-wide output column group u (A=768 -> 8 groups). Each
unit is a [4096f x 96] GEMM contribution. Units are distributed over
8 cores x 5 "slots"; a slot is (one x source chunk l) x (a fixed-width
stack of units of that l). Slot unit-counts [24, 20, 16, 10, 8] sum to
78 = 624/8 and admit an EXACT partition of the ragged triangle (column
l has (12-l)*8 units), so the SPMD program is identical on every core
while each core reads only its 5 x-chunks (5.25 MB fp16) and exactly
1/8 of all weights. Cores emit per-slot partial sums; the host
scatter-adds them into the final output.

W is quantized host-side to fp8 e3m4 (x1024 scale; subnormal-range
values rounded to {0, +-min normal} so PE flush-to-zero behavior is
irrelevant), halving HBM bytes vs fp16. x stays fp16 (the matmul
stationary operand may differ in dtype from the moving operand),
keeping quantization error at ~1.4e-2 rel. PSUM accumulates fp32;
partials drain as fp16 and the host divides by 1024.

Schedule (v2): the kernel is at the PE/DMA ridge (~105 us each), so
wall time is set by how早 the PE starts and whether supply ever lags.
 - PE p-state warmup runs off a MEMSET tile (no DMA dependency), so
   the clock ramps from ~5.8 us while the first real tiles load.
 - W kk-groups ride ONE HWDGE ring (sync) in exact consumption order,
   paced by an 8-deep rotating tag (the prefetch throttle, ~4-14 MB
   ahead, which rides through transient HBM contention from the other
   7 cores). The FIRST slot's x pieces are interleaved on the same
   ring right before the W group that needs them (deterministic
   arrival for the critical start); later slots' x chunks are whole
   sync-ring items inserted mid-previous-slot where supply slack has
   accumulated. (Putting x on the scalar ring does NOT work: HWDGE
   engine arbitration starves the minority ring when the W ring is
   saturated -- measured both as v1's 16.2 us first-x and as a
   regression when all x was scalar-ring-queued up front.)
 - Slots run wide->narrow [0,2,1,3,4]: narrow slots cost the most x
   bytes per PE-second, so they run last when DMA has slack. First
   slot's W groups ramp 2->8 kk so the first matmul starts ~9 us;
   steady groups are 8 kk so a straggling group stalls at most ~2 us.
 - Last slot loops chunk-outer so its first output chunk drains under
   the second chunk's compute; other slots drain via gpsimd SWDGE.
"""

import numpy as np
import ml_dtypes
from contextlib import ExitStack

import concourse.bass as bass
import concourse.tile as tile
from concourse import bacc, mybir
from concourse.bass_utils import run_bass_kernel_spmd

N_CORES = 8
B = 128
L = 12
F = 4096
A = 768
U = 96                      # unit width (output cols)
NU = A // U                 # 8 a-units per layer
KK = F // 128               # 32 k-chunks per source layer

W_SCALE = 1024.0            # host-side fp8 scale (power of 2)

SLOT_N = [24, 20, 16, 10, 8]          # units per slot (sum 78)
SLOT_W = [n * U for n in SLOT_N]      # cols per slot
# wide -> narrow; adjacent slots' PSUM chunks fit 8 banks
# (chunk counts s0:5 s2:3 s1:4 s3:2 s4:2 -> 5+3, 3+4, 4+2, 2+2).
SLOT_ORDER = [0, 2, 1, 3, 4]

# per-slot kk-group splits. First-processed slot ramps up so the first
# matmul's W tile is small; steady-state groups are ~1.5-2.4 MB DMAs
# (8 kk max so a straggling group costs at most ~2.2 us of PE).
SLOT_KK_GROUPS = {
    0: [3, 5, 8, 8, 8],
    2: [8, 8, 8, 8],
    1: [8, 8, 8, 8],
    3: [8, 8, 8, 8],
    4: [8, 8, 8, 8],
}
# First slot's x rides the sync ring in pieces, each emitted right
# before the first W group needing it (deterministic arrival). Later
# slots' x chunks are single sync-ring items inserted mid-previous-slot
# (X_INSERT[s] = (prev_slot, group_idx): emit x_s before that group),
# where the accumulated supply slack absorbs the 1.05 MB dent. An
# attempt to put them on the scalar ring up front regressed badly: the
# HWDGE engine share starved the sync ramp and the first real matmul
# slipped from ~10.5 us to ~17-22 us.
# [5, 27] starts the PE earliest (small first piece). An [8, 24] split
# that removes the xp1 dent between g0 and g1 (a ~2.8 us PE gap) was
# measured ~1.5 us WORSE on mean exec: the later first matmul costs
# more than the gap removal saves -- the ramp is handoff-latency-bound
# and the two trade almost exactly.
SLOT_X_SPLIT = {0: [5, 27]}
# (slot, gi, kk0, kk1): load x[slot][kk0:kk1] right before W group gi of
# the slot at that ring position. Each later slot's x splits 8+24 kk:
# the 8-kk piece a slot early, the 24-kk piece after the owning slot's
# first group (which only needs kk 0-7), spreading the supply dent.
X_INSERT = [
    (2, 0, 3, 0, 8), (2, 2, 1, 8, 32),     # x2: before s0.g3 / s2.g1
    (1, 2, 2, 0, 8), (1, 1, 1, 8, 32),     # x1: before s2.g2 / s1.g1
    (3, 1, 2, 0, 8), (3, 3, 1, 8, 32),     # x3: before s1.g2 / s3.g1
    (4, 3, 2, 0, 8), (4, 4, 1, 8, 32),     # x4: before s3.g2 / s4.g1
]
N_WARMUP_MM = 42            # scratch matmuls to ramp the PE clock
# gpsimd drain order: the drain ring is gated on the LAST W group
# being resident (see drain_gate), so the four mid-slot output drains
# (8 MB chip-wide) land after the load crunch (~95-98 us) but as early
# as possible after it -- ready-first order, s3 (whose CAST lands
# ~107 us) last, so the gpsimd queue retires well before the exec tail
# even when contention pushes everything late.
DRAIN_ORDER = [0, 2, 1, 3]

# source chunk l for (slot, core) -- the exact ragged-triangle partition
SLOT_L = [
    [8, 0, 0, 0, 0, 1, 1, 6],    # slot 0: 24 units each
    [5, 2, 2, 2, 1, 1, 3, 3],    # slot 1: 20
    [9, 5, 3, 3, 4, 4, 4, 4],    # slot 2: 16
    [7, 7, 7, 7, 5, 5, 2, 2],    # slot 3: 10
    [11, 10, 10, 9, 8, 6, 6, 6], # slot 4: 8
]

_compiled_nc = None


def _unit_assignment():
    """-> units[(slot, core)] = list of (i, u), exactly SLOT_N[slot] long."""
    pieces_by_l = {l: [] for l in range(L)}
    for s in range(len(SLOT_N)):
        for c in range(N_CORES):
            pieces_by_l[SLOT_L[s][c]].append((s, c))
    out = {}
    for l in range(L):
        units = [(i, u) for i in range(l, L) for u in range(NU)]
        acc = 0
        for (s, c) in sorted(pieces_by_l[l]):
            n = SLOT_N[s]
            out[(s, c)] = units[acc:acc + n]
            acc += n
        assert acc == len(units), (l, acc, len(units))
    return out


def _bounds(sizes: list[int]) -> list[tuple[int, int]]:
    bounds = [0]
    for n in sizes:
        bounds.append(bounds[-1] + n)
    assert bounds[-1] == KK, sizes
    return list(zip(bounds[:-1], bounds[1:]))


def _chunks(w: int) -> list[tuple[int, int]]:
    """Split w cols into <=512-wide PSUM-bank chunks."""
    out = []
    s = 0
    while s < w:
        out.append((s, min(w, s + 512)))
        s += 512
    return out


def _build():
    nc = bacc.Bacc("TRN2", target_bir_lowering=False, debug=False,
                   num_devices=N_CORES)

    xdt = mybir.dt.float16
    wdt = mybir.dt.float8e3
    xs_d = [nc.dram_tensor(f"xs{s}", [128, KK, B], xdt,
                           kind="ExternalInput").ap()
            for s in range(len(SLOT_N))]
    w_d = [nc.dram_tensor(f"w{s}", [128, KK, SLOT_W[s]], wdt,
                          kind="ExternalInput").ap()
           for s in range(len(SLOT_N))]
    out_d = [nc.dram_tensor(f"out{s}", [B, SLOT_W[s]], mybir.dt.float16,
                            kind="ExternalOutput").ap()
             for s in range(len(SLOT_N))]

    s_last = SLOT_ORDER[-1]

    with tile.TileContext(nc) as tc:
        with ExitStack() as ctx:
            xpool = ctx.enter_context(tc.tile_pool(name="x", bufs=1))
            wpool = ctx.enter_context(tc.tile_pool(name="w", bufs=8))
            opool = ctx.enter_context(tc.tile_pool(name="o", bufs=2))
            ppool = ctx.enter_context(tc.tile_pool(name="ps", bufs=8,
                                                   space="PSUM"))
            wupool = ctx.enter_context(tc.tile_pool(name="wu", bufs=1))

            # PE clock warmup off a memset tile: no DMA dependency, so
            # the ramp starts as soon as the preamble ends (~5.8 us).
            warm = wupool.tile([128, 128], wdt, tag="wu", name="warm")
            nc.gpsimd.memset(warm[:], 0.0)
            wps = ppool.tile([128, 128], mybir.dt.float32, tag="pc",
                             name="warm_ps")
            for _ in range(N_WARMUP_MM):
                nc.tensor.matmul(wps[:], warm[:], warm[:],
                                 start=True, stop=True)

            # --- Load emission: one sync HWDGE ring, exact order. ---
            # W kk-groups in consumption order; slot0's x in pieces just
            # before the group needing them; later slots' x as whole
            # chunks inserted mid-previous-slot (X_INSERT).
            s_first = SLOT_ORDER[0]
            xsplit0 = _bounds(SLOT_X_SPLIT[s_first])
            x_at = {}           # (slot, gi) -> [(xslot, kk0, kk1)]
            for xs, ps, gi, kk0, kk1 in X_INSERT:
                x_at.setdefault((ps, gi), []).append((xs, kk0, kk1))
            x_parts = {}        # slot -> [(kk0, kk1, AP)]
            wgs_by_slot = {}
            covered0 = 0
            pi0 = 0
            for s in SLOT_ORDER:
                wgs = []
                for gi, (g0, g1) in enumerate(_bounds(SLOT_KK_GROUPS[s])):
                    if s == s_first:
                        while covered0 < g1:
                            xb0, xb1 = xsplit0[pi0]
                            xp = xpool.tile([128, xb1 - xb0, B], xdt,
                                            tag=f"x{s}p{pi0}", bufs=1,
                                            name=f"x{s}p{pi0}")
                            nc.sync.dma_start(xp[:],
                                              xs_d[s][:, xb0:xb1, :])
                            x_parts.setdefault(s, []).append(
                                (xb0, xb1, xp))
                            covered0 = xb1
                            pi0 += 1
                    for xs, kk0, kk1 in x_at.get((s, gi), []):
                        xp = xpool.tile([128, kk1 - kk0, B], xdt,
                                        tag=f"x{xs}k{kk0}", bufs=1,
                                        name=f"x{xs}k{kk0}")
                        nc.sync.dma_start(xp[:], xs_d[xs][:, kk0:kk1, :])
                        x_parts.setdefault(xs, []).append((kk0, kk1, xp))
                    wg = wpool.tile([128, g1 - g0, SLOT_W[s]], wdt,
                                    tag="wg", name=f"wg{s}_{g0}")
                    nc.sync.dma_start(wg[:], w_d[s][:, g0:g1, :])
                    wgs.append(wg)
                wgs_by_slot[s] = wgs
            assert covered0 == KK and len(x_parts) == len(SLOT_N)
            for s in SLOT_ORDER:
                ks = sorted(x_parts[s])
                assert ks[0][0] == 0 and ks[-1][1] == KK, (s, ks)
                x_parts[s] = ks

            # --- Compute phase, slot by slot. ---
            ots = {}
            for si, s in enumerate(SLOT_ORDER):
                w_cols = SLOT_W[s]
                groups = _bounds(SLOT_KK_GROUPS[s])
                wgs = wgs_by_slot[s]

                def xst(kk, parts=x_parts[s]):
                    for (b0, b1, xp) in parts:
                        if kk < b1:
                            return xp[:, kk - b0, :]
                    raise AssertionError(kk)

                pcs = [ppool.tile([B, c1 - c0], mybir.dt.float32, tag="pc",
                                  name=f"pc{s}_{ci}")
                       for ci, (c0, c1) in enumerate(_chunks(w_cols))]
                # per-slot resident ot tiles (bufs=1) so mid-kernel
                # drains can be deferred past the load-crunch window
                ot = opool.tile([B, w_cols], mybir.dt.float16,
                                tag=f"ot{s}", bufs=1, name=f"ot{s}")

                if s == s_last:
                    # Deferred mid-slot drains: emitted here (before the
                    # last slot's compute) in DRAIN_ORDER; the first is
                    # gated on s3's CAST (~101 us), so all of them land
                    # after the chip-wide load crunch has ended. The
                    # tiny copy below additionally gates the gpsimd ring
                    # on the LAST W group being resident, so on a
                    # contended core the drains can never compete with
                    # the critical final loads.
                    gate = wupool.tile([128, 1], wdt, tag="gate", bufs=1,
                                       name="drain_gate")
                    nc.gpsimd.tensor_copy(gate[:], wgs[-1][:, 0, 0:1])
                    for ds in DRAIN_ORDER:
                        nc.gpsimd.dma_start(out_d[ds][:], ots[ds][:])
                    # chunk-outer: chunk 0 finishes all kk first, drains
                    # (copy + out DMA) while chunk 1 still computes. All
                    # of this slot's W groups stay resident (bufs >= 4).
                    for ci, (c0, c1) in enumerate(_chunks(w_cols)):
                        for gi, (g0, g1) in enumerate(groups):
                            wg = wgs[gi]
                            for kk in range(g0, g1):
                                nc.tensor.matmul(
                                    pcs[ci][:], xst(kk),
                                    wg[:, kk - g0, c0:c1],
                                    start=(kk == 0), stop=(kk == KK - 1),
                                )
                        nc.vector.tensor_copy(ot[:, c0:c1], pcs[ci][:])
                        # ring is drained of loads by now; HWDGE has the
                        # faster first-byte for the critical final drains
                        nc.sync.dma_start(out_d[s][:, c0:c1], ot[:, c0:c1])
                else:
                    for gi, (g0, g1) in enumerate(groups):
                        wg = wgs[gi]
                        for kk in range(g0, g1):
                            for ci, (c0, c1) in enumerate(_chunks(w_cols)):
                                nc.tensor.matmul(
                                    pcs[ci][:], xst(kk),
                                    wg[:, kk - g0, c0:c1],
                                    start=(kk == 0), stop=(kk == KK - 1),
                                )
                    for ci, (c0, c1) in enumerate(_chunks(w_cols)):
                        nc.vector.tensor_copy(ot[:, c0:c1], pcs[ci][:])
                    ots[s] = ot

    nc.compile()
    return nc


def _to_e3m4(a: np.ndarray) -> np.ndarray:
    """fp32 -> e3m4 RTN, with subnormal results re-rounded to {0, +-0.25}
    so hardware FTZ of fp8 subnormal operands cannot change the result."""
    q = np.asarray(a, dtype=ml_dtypes.float8_e3m4)
    qf = q.astype(np.float32)
    sub = np.abs(qf) < 0.25
    if sub.any():
        fix = np.where(np.abs(a) >= 0.125,
                       np.copysign(np.float32(0.25), a),
                       np.float32(0.0)).astype(ml_dtypes.float8_e3m4)
        q = np.where(sub, fix, q)
    return q


def _pack_inputs(x: np.ndarray, Ws: list[np.ndarray]):
    """Host-side shard + layout. Returns (in_maps, assignment)."""
    assign = _unit_assignment()
    # x chunks: xc[l][p, kk, b] = x[b, l, kk*128 + p], fp16
    xc = np.ascontiguousarray(
        x.reshape(B, L, KK, 128).transpose(1, 3, 2, 0)).astype(np.float16)
    in_maps = []
    for c in range(N_CORES):
        m = {}
        for s in range(len(SLOT_N)):
            l = SLOT_L[s][c]
            m[f"xs{s}"] = xc[l]
            parts = []
            for (i, u) in assign[(s, c)]:
                wl = Ws[i][l]  # [F, A] fp32
                blk = wl.reshape(KK, 128, A)[:, :, u * U:(u + 1) * U]
                parts.append(blk.transpose(1, 0, 2))  # [128, KK, 96]
            m[f"w{s}"] = _to_e3m4(
                np.concatenate(parts, axis=2) * np.float32(W_SCALE))
        in_maps.append(m)
    return in_maps, assign


def _run(inputs: dict, trace: bool = False):
    global _compiled_nc
    if _compiled_nc is None:
        _compiled_nc = _build()
    x = np.asarray(inputs["x"], dtype=np.float32)
    Ws = [np.asarray(inputs[f"W_{i}"], dtype=np.float32) for i in range(L)]
    in_maps, assign = _pack_inputs(x, Ws)
    res = run_bass_kernel_spmd(_compiled_nc, in_maps,
                               core_ids=list(range(N_CORES)), trace=trace)
    out = np.zeros((B, L, A), dtype=np.float32)
    for c in range(N_CORES):
        for s in range(len(SLOT_N)):
            part = res.results[c][f"out{s}"].astype(np.float32)
            for k, (i, u) in enumerate(assign[(s, c)]):
                out[:, i, u * U:(u + 1) * U] += part[:, k * U:(k + 1) * U]
    out *= np.float32(1.0 / W_SCALE)
    return out, res


def kernel(**inputs: np.ndarray) -> np.ndarray:
    out, _ = _run(inputs, trace=False)
    return out
